# revision 25
# baseline (speedup 1.0000x reference)
"""Adaptive average pooling (8,384,384,64) NHWC -> (8,7,7,64) on 8 TRN2 NeuronCores.

Pure data parallel: one batch sample per core, no collectives. Per core:
  - W is covered by 4 overlapping spans [0,110) [109,220) [219,330)
    [328,384); each span holds two whole adaptive W-windows (last: one).
    Spans 0-2 stream as 9 slabs (span x 3 H-chunks) via SWDGE DMAs that
    cast f32 -> bf16 in flight, alternating between two SWDGE queues.
    SWDGE leaves one SDMA engine ~23% slow (queue bookkeeping shares its
    port), which paces every completion semaphore, so span 3 streams as
    f32 over the idle sync HWDGE ring instead and the DVE downcasts its
    three small slabs early.
  - Windows are processed in order [6, 0, 1, 2, 3, 4, 5]: window 6 (span
    3) first, since its data arrives early, leaving only window 5 after
    the SWDGE stream tail.
  - TensorEngine reduces over H (the partition dim) with bf16 matmuls: for
    each W-window j and H-chunk k the stationary P_{j,k} (128 x 7) is an
    exact 0/1 bf16 membership mask of the H-windows; the moving operand is
    a CONTIGUOUS 512-column slice (8 w x 64 c) of the window's 56-wide view
    (strided rhs runs at ~2.4 cyc/row, contiguous at 1), accumulating into
    PSUM banks 0-6 as psum[i, cb, w', c] = sum_h P[h,i] * x[h, w, c].
  - ScalarEngine (ACT) drains each PSUM bank to a 2-window bf16 SBUF ring
    right after its stop-matmul, so the PE's next window never waits on a
    full-window drain.
  - DVE reduces each drained window over (cb, w') with a strided XY
    tensor_reduce, subtracts the one out-of-window column for the two
    55-wide windows (0 and 6), applies the exact fp32 1/(sh_i*sw_j)
    table, and one DMA writes the (7 x 448) result out.

Raw Bass blocks with explicit semaphores (TileContext's generated sync
exceeds this toolchain's per-instruction sync-wait limits).
"""

import numpy as np
import ml_dtypes

import concourse.bass as bass
import concourse.mybir as mybir
from concourse.bass_utils import run_bass_kernel_spmd

B, H, W, C = 8, 384, 384, 64
OUT = 7
N_CORES = 8
KH = H // 128  # 3 H-chunks of 128 rows
WMAX = 56  # uniform per-window view width along W
NCH = 7  # 512-col chunks per window
WIN = NCH * 512  # t columns per window
SPANS = [(0, 110), (109, 111), (219, 56), (274, 56), (328, 56)]  # (w0, width)
NSW = 12  # SWDGE slabs: spans 0-3 x 3 H-chunks, s = g*KH + k
SLOT = 111 * C  # SWDGE ring slot size in elements
RING = 6  # SWDGE slab ring depth
S3 = WMAX * C  # span-3 slab size in elements
# (span g, local w-offset of the 56-wide view, garbage column or None)
WINDOWS = [
    (0, 0, 55),
    (0, 54, None),
    (1, 0, None),
    (1, 55, None),
    (2, 0, None),
    (3, 0, None),
    (4, 0, 0),
]
FIRST_WIN = {0: 0, 1: 2, 2: 4, 3: 5}  # span -> its first (or only) window
LAST_WIN = {0: 1, 1: 3, 2: 4, 3: 5}  # span -> its last window
WORDER = [6, 0, 1, 2, 3, 4, 5]  # PE/ACT/DVE window processing order

_F32 = mybir.dt.float32
_BF16 = mybir.dt.bfloat16


def _windows(d, out):
    starts = np.floor(np.arange(out) * d / out).astype(np.int64)
    ends = np.ceil((np.arange(out) + 1) * d / out).astype(np.int64)
    return starts, ends - starts


def _build():
    nc = bass.Bass(num_swdge_queues=2)
    x = nc.declare_dram_parameter("x", [H, W * C], _F32, isOutput=False)
    pmat = nc.declare_dram_parameter(
        "pmat", [128, OUT * KH * OUT], _BF16, isOutput=False
    )
    invhw = nc.declare_dram_parameter("invhw", [OUT, OUT * C], _F32, isOutput=False)
    out = nc.declare_dram_parameter("out", [OUT, OUT * C], _F32, isOutput=True)

    with (
        nc.sbuf_tensor([128, RING * SLOT], _BF16) as xbuf,
        nc.sbuf_tensor([128, KH * S3], _F32) as stage3,
        nc.sbuf_tensor([128, KH * S3], _BF16) as xb3,
        nc.sbuf_tensor([128, OUT * KH * OUT], _BF16) as p_sb,
        nc.sbuf_tensor([OUT, 2 * WIN], _BF16) as t_sb,
        nc.sbuf_tensor([OUT, OUT * C], _F32) as invhw_sb,
        nc.sbuf_tensor([OUT, OUT * C], _F32) as y_raw,
        nc.sbuf_tensor([OUT, OUT * C], _F32) as y_sb,
        nc.psum_tensor([128, NCH * 512], _F32) as psum,
        nc.semaphore("const_sem") as const_sem,
        nc.semaphore("conv_sem") as conv_sem,
        nc.semaphore("pe_pass_sem") as pe_pass_sem,
        nc.semaphore("chunk_sem") as chunk_sem,
        nc.semaphore("drain_sem") as drain_sem,
        nc.semaphore("dve_sem") as dve_sem,
        nc.semaphore("mul_sem") as mul_sem,
        nc.semaphore("out_sem") as out_sem,
    ):
        slab_sems = [nc.alloc_semaphore(f"slab{s}") for s in range(NSW)]
        s3_sems = [nc.alloc_semaphore(f"s3_{k}") for k in range(KH)]

        # dve_sem ticks accumulated after the o-th processed window
        dve_ticks_after = []
        t = 0
        for o in range(OUT):
            t += 1 + (1 if WINDOWS[WORDER[o]][2] is not None else 0)
            dve_ticks_after.append(t)

        # pe_pass_sem ticks: +1 after each non-stop pass, in processing order
        def pe_pass_tick(o, k):
            return o * (KH - 1) + k + 1

        def pass_wait(eng, o, k):
            """Wait until the PE finished pass (order-index o, H-chunk k)."""
            if k == KH - 1:
                eng.wait_ge(chunk_sem, (o + 1) * NCH)
            else:
                eng.wait_ge(pe_pass_sem, pe_pass_tick(o, k))

        # order-index of each window / of a span's last window
        ORDER_OF = {j: o for o, j in enumerate(WORDER)}

        with nc.Block(no_gpsimd_drain=True) as block:

            @block.gpsimd
            def _(gpsimd):
                gpsimd.dma_start(out=p_sb[:], in_=pmat[:]).then_inc(const_sem, 16)
                gpsimd.dma_start(out=invhw_sb[:], in_=invhw[:]).then_inc(
                    const_sem, 16
                )
                for s in range(NSW):
                    g, k = divmod(s, KH)
                    w0, wd = SPANS[g]
                    r = s % RING
                    if s >= RING:
                        gp, kp = divmod(s - RING, KH)
                        pass_wait(gpsimd, ORDER_OF[LAST_WIN[gp]], kp)
                    dma = gpsimd.dma_start(
                        out=xbuf[:, r * SLOT : r * SLOT + wd * C],
                        in_=x[k * 128 : (k + 1) * 128, w0 * C : (w0 + wd) * C],
                    ).then_inc(slab_sems[s], 16)
                    if s % 2 == 1:
                        dma.ins.queue = "qPoolDynamic1"

            @block.sync
            def _(sync):
                w0, wd = SPANS[4]
                for k in range(KH):
                    sync.dma_start(
                        out=stage3[:, k * S3 : (k + 1) * S3],
                        in_=x[k * 128 : (k + 1) * 128, w0 * C : (w0 + wd) * C],
                    ).then_inc(s3_sems[k], 16)
                sync.wait_ge(mul_sem, 1)
                sync.dma_start(out=out[:], in_=y_sb[:]).then_inc(out_sem, 16)
                sync.wait_ge(out_sem, 16)

            @block.tensor
            def _(tensor):
                tensor.wait_ge(const_sem, 32)
                for o, j in enumerate(WORDER):
                    g, off, _ = WINDOWS[j]
                    for k in range(KH):
                        if g == 4:
                            tensor.wait_ge(conv_sem, k + 1)
                            src = xb3
                            base = k * S3 + off * C
                        else:
                            s = g * KH + k
                            if j == FIRST_WIN[g]:
                                tensor.wait_ge(slab_sems[s], 16)
                            src = xbuf
                            base = (s % RING) * SLOT + off * C
                        n = j * KH + k
                        lhsT = p_sb[:, n * OUT : (n + 1) * OUT]
                        for cb in range(NCH):
                            if o > 0 and k == 0:
                                tensor.wait_ge(drain_sem, (o - 1) * NCH + cb + 1)
                            mm = tensor.matmul(
                                psum[:OUT, cb * 512 : (cb + 1) * 512],
                                lhsT,
                                src[:, base + cb * 512 : base + (cb + 1) * 512],
                                start=(k == 0),
                                stop=(k == KH - 1),
                            )
                            if k == KH - 1:
                                mm.then_inc(chunk_sem, 1)
                        if k < KH - 1:
                            mm.then_inc(pe_pass_sem, 1)

            @block.scalar
            def _(scalar):
                # drain each finished PSUM bank to bf16 SBUF (2-window ring)
                for o in range(OUT):
                    t0 = (o % 2) * WIN
                    for cb in range(NCH):
                        if o >= 2 and cb == 0:
                            scalar.wait_ge(dve_sem, dve_ticks_after[o - 2])
                        scalar.wait_ge(chunk_sem, o * NCH + cb + 1)
                        scalar.copy(
                            out=t_sb[:, t0 + cb * 512 : t0 + (cb + 1) * 512],
                            in_=psum[:OUT, cb * 512 : (cb + 1) * 512],
                        ).then_inc(drain_sem, 1)

            @block.vector
            def _(vector):
                # early: downcast span 3's three f32 slabs to bf16
                for k in range(KH):
                    vector.wait_ge(s3_sems[k], 16)
                    vector.tensor_copy(
                        xb3[:, k * S3 : (k + 1) * S3],
                        stage3[:, k * S3 : (k + 1) * S3],
                    ).then_inc(conv_sem, 1)
                ticks = 0
                for o, j in enumerate(WORDER):
                    t0 = (o % 2) * WIN
                    vector.wait_ge(drain_sem, NCH * (o + 1))
                    vector.tensor_reduce(
                        out=y_raw[:, j * C : (j + 1) * C],
                        in_=t_sb[:, t0 : t0 + WIN].rearrange(
                            "p (n w c) -> p c n w", n=NCH, c=C
                        ),
                        axis=mybir.AxisListType.XY,
                        op=mybir.AluOpType.add,
                    ).then_inc(dve_sem, 1)
                    ticks += 1
                    garb = WINDOWS[j][2]
                    if garb is not None:
                        vector.wait_ge(dve_sem, ticks)
                        g0 = t0 + garb * C
                        vector.tensor_sub(
                            y_raw[:, j * C : (j + 1) * C],
                            y_raw[:, j * C : (j + 1) * C],
                            t_sb[:, g0 : g0 + C],
                        ).then_inc(dve_sem, 1)
                        ticks += 1
                vector.wait_ge(const_sem, 32)
                vector.wait_ge(dve_sem, ticks)
                vector.tensor_mul(y_sb[:], y_raw[:], invhw_sb[:]).then_inc(
                    mul_sem, 1
                )

    return nc


def _consts():
    hs, hsz = _windows(H, OUT)
    _, wsz = _windows(W, OUT)
    p = np.zeros((128, OUT * KH * OUT), np.float32)
    for j in range(OUT):
        for k in range(KH):
            n = j * KH + k
            for i in range(OUT):
                h0, h1 = int(hs[i]), int(hs[i] + hsz[i])
                for h in range(max(h0, k * 128), min(h1, (k + 1) * 128)):
                    p[h - k * 128, n * OUT + i] = 1.0
    inv = np.zeros((OUT, OUT * C), np.float32)
    for i in range(OUT):
        for j in range(OUT):
            inv[i, j * C : (j + 1) * C] = 1.0 / (float(hsz[i]) * float(wsz[j]))
    return p.astype(ml_dtypes.bfloat16), inv


_NC_CACHE = None


def _run(x, **kwargs):
    global _NC_CACHE
    if _NC_CACHE is None:
        _NC_CACHE = _build()
    nc = _NC_CACHE
    p, inv = _consts()
    x = np.ascontiguousarray(np.asarray(x, dtype=np.float32))
    in_maps = [
        {"x": x[b].reshape(H, W * C), "pmat": p, "invhw": inv}
        for b in range(N_CORES)
    ]
    res = run_bass_kernel_spmd(nc, in_maps, core_ids=list(range(N_CORES)), **kwargs)
    y = np.stack(
        [res.results[b]["out"].reshape(OUT, OUT, C) for b in range(N_CORES)]
    )
    return y, res


def kernel(x: np.ndarray) -> np.ndarray:
    y, _ = _run(x)
    return y


# revision 26
# speedup vs baseline: 1.0207x; 1.0207x over previous
"""Adaptive average pooling (8,384,384,64) NHWC -> (8,7,7,64) on 8 TRN2 NeuronCores.

Pure data parallel: one batch sample per core, no collectives. Per core:
  - W is covered by 4 overlapping spans [0,110) [109,220) [219,330)
    [328,384); each span holds two whole adaptive W-windows (last: one).
    Spans 0-2 stream as 9 slabs (span x 3 H-chunks) via SWDGE DMAs that
    cast f32 -> bf16 in flight, alternating between two SWDGE queues.
    SWDGE leaves one SDMA engine ~23% slow (queue bookkeeping shares its
    port), which paces every completion semaphore, so span 3 streams as
    f32 over the idle sync HWDGE ring instead and the DVE downcasts its
    three small slabs early.
  - Windows are processed in order [6, 0, 1, 2, 3, 4, 5]: window 6 (span
    3) first, since its data arrives early, leaving only window 5 after
    the SWDGE stream tail.
  - TensorEngine reduces over H (the partition dim) with bf16 matmuls: for
    each W-window j and H-chunk k the stationary P_{j,k} (128 x 7) is an
    exact 0/1 bf16 membership mask of the H-windows; the moving operand is
    a CONTIGUOUS 512-column slice (8 w x 64 c) of the window's 56-wide view
    (strided rhs runs at ~2.4 cyc/row, contiguous at 1), accumulating into
    PSUM banks 0-6 as psum[i, cb, w', c] = sum_h P[h,i] * x[h, w, c].
  - ScalarEngine (ACT) drains each PSUM bank to a 2-window bf16 SBUF ring
    right after its stop-matmul, so the PE's next window never waits on a
    full-window drain.
  - DVE reduces each drained window over (cb, w') with a strided XY
    tensor_reduce, subtracts the one out-of-window column for the two
    55-wide windows (0 and 6), applies the exact fp32 1/(sh_i*sw_j)
    table, and one DMA writes the (7 x 448) result out.

Raw Bass blocks with explicit semaphores (TileContext's generated sync
exceeds this toolchain's per-instruction sync-wait limits).
"""

import numpy as np
import ml_dtypes

import concourse.bass as bass
import concourse.mybir as mybir
from concourse.bass_utils import run_bass_kernel_spmd

B, H, W, C = 8, 384, 384, 64
OUT = 7
N_CORES = 8
KH = H // 128  # 3 H-chunks of 128 rows
WMAX = 56  # uniform per-window view width along W
NCH = 7  # 512-col chunks per window
WIN = NCH * 512  # t columns per window
SPANS = [(0, 110), (109, 111), (219, 56), (274, 56), (328, 56)]  # (w0, width)
NSW = 12  # SWDGE slabs: spans 0-3 x 3 H-chunks, s = g*KH + k
SLOT = 111 * C  # SWDGE ring slot size in elements
RING = 6  # SWDGE slab ring depth
S3 = WMAX * C  # span-3 slab size in elements
# (span g, local w-offset of the 56-wide view, garbage column or None)
WINDOWS = [
    (0, 0, 55),
    (0, 54, None),
    (1, 0, None),
    (1, 55, None),
    (2, 0, None),
    (3, 0, None),
    (4, 0, 0),
]
FIRST_WIN = {0: 0, 1: 2, 2: 4, 3: 5}  # span -> its first (or only) window
LAST_WIN = {0: 1, 1: 3, 2: 4, 3: 5}  # span -> its last window
WORDER = [6, 0, 1, 2, 3, 4, 5]  # PE/ACT/DVE window processing order

_F32 = mybir.dt.float32
_BF16 = mybir.dt.bfloat16


def _windows(d, out):
    starts = np.floor(np.arange(out) * d / out).astype(np.int64)
    ends = np.ceil((np.arange(out) + 1) * d / out).astype(np.int64)
    return starts, ends - starts


def _build():
    nc = bass.Bass(num_swdge_queues=2)
    x = nc.declare_dram_parameter("x", [H, W * C], _F32, isOutput=False)
    pmat = nc.declare_dram_parameter(
        "pmat", [128, OUT * KH * OUT], _BF16, isOutput=False
    )
    invhw = nc.declare_dram_parameter("invhw", [OUT, OUT * C], _F32, isOutput=False)
    out = nc.declare_dram_parameter("out", [OUT, OUT * C], _F32, isOutput=True)

    with (
        nc.sbuf_tensor([128, RING * SLOT], _BF16) as xbuf,
        nc.sbuf_tensor([128, KH * S3], _F32) as stage3,
        nc.sbuf_tensor([128, KH * S3], _BF16) as xb3,
        nc.sbuf_tensor([128, OUT * KH * OUT], _BF16) as p_sb,
        nc.sbuf_tensor([OUT, 2 * WIN], _BF16) as t_sb,
        nc.sbuf_tensor([OUT, OUT * C], _F32) as invhw_sb,
        nc.sbuf_tensor([OUT, OUT * C], _F32) as y_raw,
        nc.sbuf_tensor([OUT, OUT * C], _F32) as y_sb,
        nc.psum_tensor([128, NCH * 512], _F32) as psum,
        nc.semaphore("const_sem") as const_sem,
        nc.semaphore("conv_sem") as conv_sem,
        nc.semaphore("pe_pass_sem") as pe_pass_sem,
        nc.semaphore("chunk_sem") as chunk_sem,
        nc.semaphore("drain_sem") as drain_sem,
        nc.semaphore("dve_sem") as dve_sem,
        nc.semaphore("mul_sem") as mul_sem,
        nc.semaphore("out_sem") as out_sem,
    ):
        slab_sems = [nc.alloc_semaphore(f"slab{s}") for s in range(NSW)]
        s3_sems = [nc.alloc_semaphore(f"s3_{k}") for k in range(KH)]

        # dve_sem ticks accumulated after the o-th processed window
        dve_ticks_after = []
        t = 0
        for o in range(OUT):
            t += 1 + (1 if WINDOWS[WORDER[o]][2] is not None else 0)
            dve_ticks_after.append(t)

        # pe_pass_sem ticks: +1 after each non-stop pass, in processing order
        def pe_pass_tick(o, k):
            return o * (KH - 1) + k + 1

        def pass_wait(eng, o, k):
            """Wait until the PE finished pass (order-index o, H-chunk k)."""
            if k == KH - 1:
                eng.wait_ge(chunk_sem, (o + 1) * NCH)
            else:
                eng.wait_ge(pe_pass_sem, pe_pass_tick(o, k))

        # order-index of each window / of a span's last window
        ORDER_OF = {j: o for o, j in enumerate(WORDER)}

        with nc.Block() as block:

            @block.gpsimd
            def _(gpsimd):
                gpsimd.dma_start(out=p_sb[:], in_=pmat[:]).then_inc(const_sem, 16)
                gpsimd.dma_start(out=invhw_sb[:], in_=invhw[:]).then_inc(
                    const_sem, 16
                )
                for s in range(NSW):
                    g, k = divmod(s, KH)
                    w0, wd = SPANS[g]
                    r = s % RING
                    if s >= RING:
                        gp, kp = divmod(s - RING, KH)
                        pass_wait(gpsimd, ORDER_OF[LAST_WIN[gp]], kp)
                    dma = gpsimd.dma_start(
                        out=xbuf[:, r * SLOT : r * SLOT + wd * C],
                        in_=x[k * 128 : (k + 1) * 128, w0 * C : (w0 + wd) * C],
                    ).then_inc(slab_sems[s], 16)
                    if s % 2 == 1:
                        dma.ins.queue = "qPoolDynamic1"

            @block.sync
            def _(sync):
                w0, wd = SPANS[4]
                for k in range(KH):
                    sync.dma_start(
                        out=stage3[:, k * S3 : (k + 1) * S3],
                        in_=x[k * 128 : (k + 1) * 128, w0 * C : (w0 + wd) * C],
                    ).then_inc(s3_sems[k], 16)
                sync.wait_ge(mul_sem, 1)
                sync.dma_start(out=out[:], in_=y_sb[:]).then_inc(out_sem, 16)
                sync.wait_ge(out_sem, 16)

            @block.tensor
            def _(tensor):
                tensor.wait_ge(const_sem, 32)
                for o, j in enumerate(WORDER):
                    g, off, _ = WINDOWS[j]
                    for k in range(KH):
                        if g == 4:
                            tensor.wait_ge(conv_sem, k + 1)
                            src = xb3
                            base = k * S3 + off * C
                        else:
                            s = g * KH + k
                            if j == FIRST_WIN[g]:
                                tensor.wait_ge(slab_sems[s], 16)
                            src = xbuf
                            base = (s % RING) * SLOT + off * C
                        n = j * KH + k
                        lhsT = p_sb[:, n * OUT : (n + 1) * OUT]
                        for cb in range(NCH):
                            if o > 0 and k == 0:
                                tensor.wait_ge(drain_sem, (o - 1) * NCH + cb + 1)
                            mm = tensor.matmul(
                                psum[:OUT, cb * 512 : (cb + 1) * 512],
                                lhsT,
                                src[:, base + cb * 512 : base + (cb + 1) * 512],
                                start=(k == 0),
                                stop=(k == KH - 1),
                            )
                            if k == KH - 1:
                                mm.then_inc(chunk_sem, 1)
                        if k < KH - 1:
                            mm.then_inc(pe_pass_sem, 1)

            @block.scalar
            def _(scalar):
                # drain each finished PSUM bank to bf16 SBUF (2-window ring)
                for o in range(OUT):
                    t0 = (o % 2) * WIN
                    for cb in range(NCH):
                        if o >= 2 and cb == 0:
                            scalar.wait_ge(dve_sem, dve_ticks_after[o - 2])
                        scalar.wait_ge(chunk_sem, o * NCH + cb + 1)
                        scalar.copy(
                            out=t_sb[:, t0 + cb * 512 : t0 + (cb + 1) * 512],
                            in_=psum[:OUT, cb * 512 : (cb + 1) * 512],
                        ).then_inc(drain_sem, 1)

            @block.vector
            def _(vector):
                # early: downcast span 3's three f32 slabs to bf16
                for k in range(KH):
                    vector.wait_ge(s3_sems[k], 16)
                    vector.tensor_copy(
                        xb3[:, k * S3 : (k + 1) * S3],
                        stage3[:, k * S3 : (k + 1) * S3],
                    ).then_inc(conv_sem, 1)
                ticks = 0
                for o, j in enumerate(WORDER):
                    t0 = (o % 2) * WIN
                    vector.wait_ge(drain_sem, NCH * (o + 1))
                    vector.tensor_reduce(
                        out=y_raw[:, j * C : (j + 1) * C],
                        in_=t_sb[:, t0 : t0 + WIN].rearrange(
                            "p (n w c) -> p c n w", n=NCH, c=C
                        ),
                        axis=mybir.AxisListType.XY,
                        op=mybir.AluOpType.add,
                    ).then_inc(dve_sem, 1)
                    ticks += 1
                    garb = WINDOWS[j][2]
                    if garb is not None:
                        vector.wait_ge(dve_sem, ticks)
                        g0 = t0 + garb * C
                        vector.tensor_sub(
                            y_raw[:, j * C : (j + 1) * C],
                            y_raw[:, j * C : (j + 1) * C],
                            t_sb[:, g0 : g0 + C],
                        ).then_inc(dve_sem, 1)
                        ticks += 1
                vector.wait_ge(const_sem, 32)
                vector.wait_ge(dve_sem, ticks)
                vector.tensor_mul(y_sb[:], y_raw[:], invhw_sb[:]).then_inc(
                    mul_sem, 1
                )

    return nc


def _consts():
    hs, hsz = _windows(H, OUT)
    _, wsz = _windows(W, OUT)
    p = np.zeros((128, OUT * KH * OUT), np.float32)
    for j in range(OUT):
        for k in range(KH):
            n = j * KH + k
            for i in range(OUT):
                h0, h1 = int(hs[i]), int(hs[i] + hsz[i])
                for h in range(max(h0, k * 128), min(h1, (k + 1) * 128)):
                    p[h - k * 128, n * OUT + i] = 1.0
    inv = np.zeros((OUT, OUT * C), np.float32)
    for i in range(OUT):
        for j in range(OUT):
            inv[i, j * C : (j + 1) * C] = 1.0 / (float(hsz[i]) * float(wsz[j]))
    return p.astype(ml_dtypes.bfloat16), inv


_NC_CACHE = None


def _run(x, **kwargs):
    global _NC_CACHE
    if _NC_CACHE is None:
        _NC_CACHE = _build()
    nc = _NC_CACHE
    p, inv = _consts()
    x = np.ascontiguousarray(np.asarray(x, dtype=np.float32))
    in_maps = [
        {"x": x[b].reshape(H, W * C), "pmat": p, "invhw": inv}
        for b in range(N_CORES)
    ]
    res = run_bass_kernel_spmd(nc, in_maps, core_ids=list(range(N_CORES)), **kwargs)
    y = np.stack(
        [res.results[b]["out"].reshape(OUT, OUT, C) for b in range(N_CORES)]
    )
    return y, res


def kernel(x: np.ndarray) -> np.ndarray:
    y, _ = _run(x)
    return y


# revision 29
# speedup vs baseline: 1.0907x; 1.0686x over previous
"""Adaptive average pooling (8,384,384,64) NHWC -> (8,7,7,64) on 8 TRN2 NeuronCores.

Pure data parallel: one batch sample per core, no collectives. Per core:
  - W is covered by 5 overlapping spans [0,110) [109,220) [219,275)
    [274,330) [328,384); spans 0-1 hold two adaptive W-windows each, spans
    2-4 one. Spans 0-3 stream as 12 slabs (span x 3 H-chunks) via SWDGE
    DMAs that cast f32 -> bf16 in flight (alternating two SWDGE queues);
    span 4 streams f32 over the idle sync HWDGE ring and the DVE downcasts
    its three slabs early. Windows are processed [6, 0, 1, 2, 3, 4, 5] so
    only single-window spans remain after the SWDGE stream tail.
  - TensorEngine reduces over H (the partition dim) with bf16 matmuls: for
    each W-window j and H-chunk k the stationary P_{j,k} (128 x 7) is an
    exact 0/1 bf16 membership mask of the H-windows; the moving operand is
    a CONTIGUOUS 512-column slice (8 w x 64 c) of the window's 56-wide view
    (strided rhs runs at ~2.4 cyc/row, contiguous at 1), accumulating into
    PSUM banks 0-6 as psum[i, cb, w', c] = sum_h P[h,i] * x[h, w, c].
  - DVE reduces each PSUM bank over w' right after its stop-matmul
    (partial per bank, strided X reduce of 512 elements), then combines
    the 7 partials, subtracts the one out-of-window column for the two
    55-wide windows (copied out of PSUM before the bank is released),
    applies the exact fp32 1/(sh_i*sw_j) table, and one DMA writes the
    (7 x 448) result out.

Raw Bass blocks with explicit semaphores (TileContext's generated sync
exceeds this toolchain's per-instruction sync-wait limits).
"""

import numpy as np
import ml_dtypes

import concourse.bass as bass
import concourse.mybir as mybir
from concourse.bass_utils import run_bass_kernel_spmd

B, H, W, C = 8, 384, 384, 64
OUT = 7
N_CORES = 8
KH = H // 128  # 3 H-chunks of 128 rows
WMAX = 56  # uniform per-window view width along W
NCH = 7  # 512-col chunks per window
SPANS = [(0, 110), (109, 111), (219, 56), (274, 56), (328, 56)]  # (w0, width)
NSW = 12  # SWDGE slabs: spans 0-3 x 3 H-chunks, s = g*KH + k
SLOT = 111 * C  # SWDGE ring slot size in elements
RING = 6  # SWDGE slab ring depth
S3 = WMAX * C  # span-4 slab size in elements
# (span g, local w-offset of the 56-wide view, garbage view-column or None)
WINDOWS = [
    (0, 0, 55),
    (0, 54, None),
    (1, 0, None),
    (1, 55, None),
    (2, 0, None),
    (3, 0, None),
    (4, 0, 0),
]
FIRST_WIN = {0: 0, 1: 2, 2: 4, 3: 5}  # SWDGE span -> its first window
LAST_WIN = {0: 1, 1: 3, 2: 4, 3: 5}  # SWDGE span -> its last window
WORDER = [6, 0, 1, 2, 3, 4, 5]  # PE/DVE window processing order

_F32 = mybir.dt.float32
_BF16 = mybir.dt.bfloat16


def _windows(d, out):
    starts = np.floor(np.arange(out) * d / out).astype(np.int64)
    ends = np.ceil((np.arange(out) + 1) * d / out).astype(np.int64)
    return starts, ends - starts


def _build():
    nc = bass.Bass(num_swdge_queues=2)
    x = nc.declare_dram_parameter("x", [H, W * C], _F32, isOutput=False)
    pmat = nc.declare_dram_parameter(
        "pmat", [128, OUT * KH * OUT], _BF16, isOutput=False
    )
    invhw = nc.declare_dram_parameter("invhw", [OUT, OUT * C], _F32, isOutput=False)
    out = nc.declare_dram_parameter("out", [OUT, OUT * C], _F32, isOutput=True)

    with (
        nc.sbuf_tensor([128, RING * SLOT], _BF16) as xbuf,
        nc.sbuf_tensor([128, KH * S3], _F32) as stage3,
        nc.sbuf_tensor([128, KH * S3], _BF16) as xb3,
        nc.sbuf_tensor([128, OUT * KH * OUT], _BF16) as p_sb,
        nc.sbuf_tensor([OUT, NCH * C], _F32) as part_sb,
        nc.sbuf_tensor([OUT, C], _F32) as garb_sb,
        nc.sbuf_tensor([OUT, OUT * C], _F32) as invhw_sb,
        nc.sbuf_tensor([OUT, OUT * C], _F32) as y_raw,
        nc.sbuf_tensor([OUT, OUT * C], _F32) as y_sb,
        nc.psum_tensor([128, NCH * 512], _F32) as psum,
        nc.semaphore("const_sem") as const_sem,
        nc.semaphore("conv_sem") as conv_sem,
        nc.semaphore("pe_pass_sem") as pe_pass_sem,
        nc.semaphore("chunk_sem") as chunk_sem,
        nc.semaphore("part_sem") as part_sem,
        nc.semaphore("dve_sem") as dve_sem,
        nc.semaphore("mul_sem") as mul_sem,
        nc.semaphore("out_sem") as out_sem,
    ):
        slab_sems = [nc.alloc_semaphore(f"slab{s}") for s in range(NSW)]
        s3_sems = [nc.alloc_semaphore(f"s3_{k}") for k in range(KH)]

        ORDER_OF = {j: o for o, j in enumerate(WORDER)}

        def pass_wait(eng, o, k):
            """Wait until the PE finished pass (order-index o, H-chunk k)."""
            if k == KH - 1:
                eng.wait_ge(chunk_sem, (o + 1) * NCH)
            else:
                eng.wait_ge(pe_pass_sem, o * (KH - 1) + k + 1)

        with nc.Block() as block:

            @block.gpsimd
            def _(gpsimd):
                gpsimd.dma_start(out=p_sb[:], in_=pmat[:]).then_inc(const_sem, 16)
                gpsimd.dma_start(out=invhw_sb[:], in_=invhw[:]).then_inc(
                    const_sem, 16
                )
                for s in range(NSW):
                    g, k = divmod(s, KH)
                    w0, wd = SPANS[g]
                    r = s % RING
                    if s >= RING:
                        gp, kp = divmod(s - RING, KH)
                        pass_wait(gpsimd, ORDER_OF[LAST_WIN[gp]], kp)
                    dma = gpsimd.dma_start(
                        out=xbuf[:, r * SLOT : r * SLOT + wd * C],
                        in_=x[k * 128 : (k + 1) * 128, w0 * C : (w0 + wd) * C],
                    ).then_inc(slab_sems[s], 16)
                    if s % 2 == 1:
                        dma.ins.queue = "qPoolDynamic1"

            @block.sync
            def _(sync):
                w0, wd = SPANS[4]
                for k in range(KH):
                    sync.dma_start(
                        out=stage3[:, k * S3 : (k + 1) * S3],
                        in_=x[k * 128 : (k + 1) * 128, w0 * C : (w0 + wd) * C],
                    ).then_inc(s3_sems[k], 16)
                sync.wait_ge(mul_sem, 1)
                sync.dma_start(out=out[:], in_=y_sb[:]).then_inc(out_sem, 16)
                sync.wait_ge(out_sem, 16)

            @block.tensor
            def _(tensor):
                tensor.wait_ge(const_sem, 32)
                for o, j in enumerate(WORDER):
                    g, off, _ = WINDOWS[j]
                    for k in range(KH):
                        if g == 4:
                            tensor.wait_ge(conv_sem, k + 1)
                            src = xb3
                            base = k * S3 + off * C
                        else:
                            s = g * KH + k
                            if j == FIRST_WIN[g]:
                                tensor.wait_ge(slab_sems[s], 16)
                            src = xbuf
                            base = (s % RING) * SLOT + off * C
                        n = j * KH + k
                        lhsT = p_sb[:, n * OUT : (n + 1) * OUT]
                        for cb in range(NCH):
                            if o > 0 and k == 0:
                                # WAR: previous window's bank cb reduced
                                tensor.wait_ge(part_sem, (o - 1) * NCH + cb + 1)
                            mm = tensor.matmul(
                                psum[:OUT, cb * 512 : (cb + 1) * 512],
                                lhsT,
                                src[:, base + cb * 512 : base + (cb + 1) * 512],
                                start=(k == 0),
                                stop=(k == KH - 1),
                            )
                            if k == KH - 1:
                                mm.then_inc(chunk_sem, 1)
                        if k < KH - 1:
                            mm.then_inc(pe_pass_sem, 1)

            @block.vector
            def _(vector):
                # early: downcast span 4's three f32 slabs to bf16
                for k in range(KH):
                    vector.wait_ge(s3_sems[k], 16)
                    vector.tensor_copy(
                        xb3[:, k * S3 : (k + 1) * S3],
                        stage3[:, k * S3 : (k + 1) * S3],
                    ).then_inc(conv_sem, 1)
                ticks = 0
                for o, j in enumerate(WORDER):
                    garb = WINDOWS[j][2]
                    gchunk = None if garb is None else garb // 8
                    for cb in range(NCH):
                        if o > 0 and cb == 0:
                            # self-wait: previous window's combine must have
                            # consumed part_sb before we overwrite it
                            vector.wait_ge(dve_sem, ticks)
                        vector.wait_ge(chunk_sem, o * NCH + cb + 1)
                        if cb == gchunk:
                            # save the out-of-window column before this bank
                            # is handed back to the PE (self-wait: the prior
                            # garbage subtract must have consumed garb_sb)
                            vector.wait_ge(dve_sem, ticks)
                            g0 = cb * 512 + (garb % 8) * C
                            vector.tensor_copy(
                                garb_sb[:], psum[:OUT, g0 : g0 + C]
                            ).then_inc(dve_sem, 1)
                            ticks += 1
                        # partial: sum bank cb over its 8 w' columns
                        vector.tensor_reduce(
                            out=part_sb[:, cb * C : (cb + 1) * C],
                            in_=psum[:OUT, cb * 512 : (cb + 1) * 512].rearrange(
                                "p (w c) -> p c w", c=C
                            ),
                            axis=mybir.AxisListType.X,
                            op=mybir.AluOpType.add,
                        ).then_inc(part_sem, 1)
                    # combine the 7 partials into the window's 64 channels
                    vector.wait_ge(part_sem, (o + 1) * NCH)
                    vector.tensor_reduce(
                        out=y_raw[:, j * C : (j + 1) * C],
                        in_=part_sb[:].rearrange("p (n c) -> p c n", c=C),
                        axis=mybir.AxisListType.X,
                        op=mybir.AluOpType.add,
                    ).then_inc(dve_sem, 1)
                    ticks += 1
                    if garb is not None:
                        vector.wait_ge(dve_sem, ticks)
                        vector.tensor_sub(
                            y_raw[:, j * C : (j + 1) * C],
                            y_raw[:, j * C : (j + 1) * C],
                            garb_sb[:],
                        ).then_inc(dve_sem, 1)
                        ticks += 1
                vector.wait_ge(const_sem, 32)
                vector.wait_ge(dve_sem, ticks)
                vector.tensor_mul(y_sb[:], y_raw[:], invhw_sb[:]).then_inc(
                    mul_sem, 1
                )

    return nc


def _consts():
    hs, hsz = _windows(H, OUT)
    _, wsz = _windows(W, OUT)
    p = np.zeros((128, OUT * KH * OUT), np.float32)
    for j in range(OUT):
        for k in range(KH):
            n = j * KH + k
            for i in range(OUT):
                h0, h1 = int(hs[i]), int(hs[i] + hsz[i])
                for h in range(max(h0, k * 128), min(h1, (k + 1) * 128)):
                    p[h - k * 128, n * OUT + i] = 1.0
    inv = np.zeros((OUT, OUT * C), np.float32)
    for i in range(OUT):
        for j in range(OUT):
            inv[i, j * C : (j + 1) * C] = 1.0 / (float(hsz[i]) * float(wsz[j]))
    return p.astype(ml_dtypes.bfloat16), inv


_NC_CACHE = None


def _run(x, **kwargs):
    global _NC_CACHE
    if _NC_CACHE is None:
        _NC_CACHE = _build()
    nc = _NC_CACHE
    p, inv = _consts()
    x = np.ascontiguousarray(np.asarray(x, dtype=np.float32))
    in_maps = [
        {"x": x[b].reshape(H, W * C), "pmat": p, "invhw": inv}
        for b in range(N_CORES)
    ]
    res = run_bass_kernel_spmd(nc, in_maps, core_ids=list(range(N_CORES)), **kwargs)
    y = np.stack(
        [res.results[b]["out"].reshape(OUT, OUT, C) for b in range(N_CORES)]
    )
    return y, res


def kernel(x: np.ndarray) -> np.ndarray:
    y, _ = _run(x)
    return y
